# revision 1
# baseline (speedup 1.0000x reference)
"""nn_DTW kernel for 8 Trainium2 NeuronCores (batch data-parallel).

See _build_cfg for the device program; the host does the sequential
backtrack pointer-chase and the final logsumexp combine.
"""

from contextlib import ExitStack

import concourse.bass as bass
import concourse.bacc as bacc
import concourse.tile as tile
from concourse import mybir
from concourse.masks import make_identity

F32 = mybir.dt.float32
AX = mybir.AxisListType
OP = mybir.AluOpType
ACT = mybir.ActivationFunctionType

BIG = 1.0e30


def _build_cfg(B=8, N=512, M=512, D=256, S=16, W=32, R=8, PART=128):
    assert S * W == M and N % R == 0
    P = S * B
    assert P <= PART
    NT = (N + PART - 1) // PART
    MT = (M + PART - 1) // PART
    DB = (D + PART - 1) // PART
    PN = min(PART, N)
    PD = min(PART, D)
    NSTEP = N // R
    T_TOT = NSTEP + S - 1
    SLOTS = N + R * S
    SLOT = W + 1

    nc = bacc.Bacc("TRN2", target_bir_lowering=False, debug=False)

    x_in = nc.dram_tensor("x", [B, N, D], F32, kind="ExternalInput").ap()
    y_in = nc.dram_tensor("y", [B, M, D], F32, kind="ExternalInput").ap()
    tc_out = nc.dram_tensor("tc_out", [P, SLOTS, SLOT], F32, kind="ExternalOutput").ap()
    neg_out = nc.dram_tensor("neg_out", [B, 1], F32, kind="ExternalOutput").ap()
    cost_stage = nc.dram_tensor("cost_stage", [NT, B, PN, M], F32).ap()

    with tile.TileContext(nc) as tcx, ExitStack() as ctx:
        const = ctx.enter_context(tcx.tile_pool(name="const", bufs=1))
        ident = const.tile([PART, PART], F32)
        make_identity(nc, ident[:])
        oneh = const.tile([PN, B, B], F32)
        nc.vector.memset(oneh[:], 0.0)
        for b_ in range(B):
            nc.vector.memset(oneh[:, b_, b_:b_ + 1], 1.0)
        big_m0 = const.tile([P, W], F32)
        nc.vector.memset(big_m0[:], BIG)
        shift8 = const.tile([PART, PART], F32)
        nc.gpsimd.memset(shift8[:], 0.0)
        nc.gpsimd.affine_select(
            out=shift8[:], in_=shift8[:], compare_op=OP.not_equal, fill=1.0,
            base=B, pattern=[[-1, PART]], channel_multiplier=1,
        )
        bigrow = const.tile([1, PART], F32)
        nc.vector.memset(bigrow[:], 0.0)
        nc.vector.memset(bigrow[0:1, 0:B], BIG)
        onesR = const.tile([1, R], F32)
        nc.vector.memset(onesR[:], 1.0)

        strip = ctx.enter_context(tcx.tile_pool(name="strip", bufs=1))
        tc_strip = strip.tile([P, SLOTS, SLOT], F32)
        nc.gpsimd.memset(tc_strip[:, :, :], BIG)

        # persistent transposed operands + per-batch scales
        oper = ctx.enter_context(tcx.tile_pool(name="oper", bufs=1))
        xTall = oper.tile([PD, B, DB, N], F32)
        ynTall = oper.tile([PD, B, DB, M], F32)
        xrn_all = oper.tile([PN, B, NT], F32)

        # stage-B pools created up-front so B0 can interleave with stage A
        stage = ctx.enter_context(tcx.tile_pool(name="stage", bufs=3))
        neg_pool = ctx.enter_context(tcx.tile_pool(name="negp", bufs=1))
        ps_c = ctx.enter_context(tcx.tile_pool(name="ps_c", bufs=3, space="PSUM"))
        ps_neg = ctx.enter_context(tcx.tile_pool(name="ps_neg", bufs=1, space="PSUM"))
        ngb = ps_neg.tile([B, M], F32, tag="ngb", bufs=1)

        def emit_stageB_batch(nt, b):
            rows = min(PART, N - nt * PART)
            psc = ps_c.tile([PN, M], F32, tag="psc", name=f"psc_{nt}_{b}")
            for db in range(DB):
                dcols = min(PART, D - db * PART)
                nc.tensor.matmul(
                    psc[:rows, :],
                    xTall[:dcols, b, db, nt * PART:nt * PART + rows],
                    ynTall[:dcols, b, db, :],
                    start=(db == 0), stop=(db == DB - 1),
                )
            cn = stage.tile([PN, M], F32, tag="cn", name=f"cn_{nt}_{b}")
            nc.scalar.activation(cn[:rows], psc[:rows], ACT.Copy,
                                 scale=xrn_all[:rows, b, nt:nt + 1], bias=1.0)
            nc.tensor.matmul(
                ngb[:, :],
                oneh[:rows, b, :],
                cn[:rows, :],
                start=(nt == 0 and b == 0),
                stop=(nt == NT - 1 and b == B - 1),
                skip_group_check=True,
            )
            heng = nc.scalar if b % 2 == 0 else nc.sync
            heng.dma_start(out=cost_stage[nt, b], in_=cn[:rows, :])

        def emit_hop2_part(nt, quarter):
            rows = min(PART, N - nt * PART)
            for s in range(quarter * S // 4, (quarter + 1) * S // 4):
                src = cost_stage[nt, :, :, s * W:(s + 1) * W]
                eng = nc.sync if s % 2 == 0 else nc.scalar
                eng.dma_start(
                    out=tc_strip[s * B:s * B + B,
                                 R * s + nt * PART:R * s + nt * PART + rows,
                                 1:SLOT],
                    in_=src)

        def emit_hop2(nt):
            rows = min(PART, N - nt * PART)
            for s in range(S):
                src = cost_stage[nt, :, :, s * W:(s + 1) * W]
                eng = nc.sync if s % 2 == 0 else nc.scalar
                eng.dma_start(
                    out=tc_strip[s * B:s * B + B,
                                 R * s + nt * PART:R * s + nt * PART + rows,
                                 1:SLOT],
                    in_=src)

        def emit_stageB(nt):
            for b in range(B):
                emit_stageB_batch(nt, b)
            emit_hop2(nt)

        # ---------------- Stage A: loads, norms, transposes ----------------
        with ExitStack() as ctxA:
            xy = ctxA.enter_context(tcx.tile_pool(name="xy", bufs=2))
            nrm = ctxA.enter_context(tcx.tile_pool(name="nrm", bufs=3))
            ps_t = ctxA.enter_context(tcx.tile_pool(name="ps_t", bufs=2, space="PSUM"))

            for b in range(B):
                y_all = xy.tile([PN, MT, D], F32, tag="ldy")
                nc.sync.dma_start(
                    out=y_all[:, :, :],
                    in_=y_in[b].rearrange("(t n) d -> n t d", t=MT))
                x_all = xy.tile([PN, NT, D], F32, tag="ldx")
                nc.sync.dma_start(
                    out=x_all[:, :, :],
                    in_=x_in[b].rearrange("(t n) d -> n t d", t=NT))

                ps_y = [ps_t.tile([PD, M], F32, tag=f"pstr{db}", name=f"psy{db}_{b}")
                        for db in range(DB)]
                for mt in range(MT):
                    rows = min(PART, M - mt * PART)
                    yt = y_all[:rows, mt, :]
                    sq = xy.tile([PART, D], F32, tag="sq")
                    s2 = nrm.tile([PART, 1], F32, tag="s2")
                    nc.scalar.activation(sq[:rows], yt, ACT.Square, accum_out=s2[:rows])
                    nrm_t = nrm.tile([PART, 1], F32, tag="nrm")
                    nc.scalar.activation(nrm_t[:rows], s2[:rows], ACT.Sqrt)
                    rn = nrm.tile([PART, 1], F32, tag="rn")
                    nc.vector.reciprocal(rn[:rows], nrm_t[:rows])
                    yn = xy.tile([PART, D], F32, tag="yn")
                    nc.vector.tensor_scalar_mul(yn[:rows], yt, rn[:rows])
                    for db in range(DB):
                        dcols = min(PART, D - db * PART)
                        nc.tensor.transpose(
                            ps_y[db][:dcols, mt * PART:mt * PART + rows],
                            yn[:rows, db * PART:db * PART + dcols],
                            ident[:rows, :rows])
                for db in range(DB):
                    nc.scalar.copy(ynTall[:, b, db, :], ps_y[db][:, :])

                ps_x = [ps_t.tile([PD, N], F32, tag=f"pstr{db}", name=f"psx{db}_{b}")
                        for db in range(DB)]
                for nt in range(NT):
                    rows = min(PART, N - nt * PART)
                    xt = x_all[:rows, nt, :]
                    sq = xy.tile([PART, D], F32, tag="sq")
                    s2 = nrm.tile([PART, 1], F32, tag="s2")
                    nc.scalar.activation(sq[:rows], xt, ACT.Square, accum_out=s2[:rows])
                    nrm_t = nrm.tile([PART, 1], F32, tag="nrm")
                    nc.scalar.activation(nrm_t[:rows], s2[:rows], ACT.Sqrt)
                    rn = nrm.tile([PART, 1], F32, tag="rn")
                    nc.vector.reciprocal(rn[:rows], nrm_t[:rows])
                    nc.vector.tensor_scalar_mul(xrn_all[:rows, b, nt:nt + 1],
                                                rn[:rows], -1.0)
                    for db in range(DB):
                        dcols = min(PART, D - db * PART)
                        nc.tensor.transpose(
                            ps_x[db][:dcols, nt * PART:nt * PART + rows],
                            xt[:, db * PART:db * PART + dcols],
                            ident[:rows, :rows])
                for db in range(DB):
                    nc.scalar.copy(xTall[:, b, db, :], ps_x[db][:, :])
                # interleave the first row-block's cost work for this batch
                emit_stageB_batch(0, b)
            emit_hop2(0)

        # ---------------- Stage C: skew-R DTW wavefront ----------------
        ps_carry = ctx.enter_context(tcx.tile_pool(name="ps_cr", bufs=1, space="PSUM"))
        mpool = ctx.enter_context(tcx.tile_pool(name="mpool", bufs=8))

        NCARRY = 4
        carry_tiles = [
            ps_carry.tile([P, R], F32, tag=f"cr{i}", name=f"carry{i}")
            for i in range(NCARRY)
        ]

        def emit_carry(U, c0, c1):
            base = R * U
            bnd = min(S - 1, U + 1) * B
            cps = carry_tiles[(U + 1) % NCARRY]
            nc.tensor.matmul(
                cps[0:bnd + B, c0:c1],
                shift8[0:bnd, 0:bnd + B],
                tc_strip[0:bnd, base + c0:base + c1, SLOT - 1:SLOT],
                start=True, stop=False, skip_group_check=True,
            )
            nc.tensor.matmul(
                cps[0:bnd + B, c0:c1],
                bigrow[0:1, 0:bnd + B],
                onesR[0:1, 0:c1 - c0],
                start=False, stop=True, skip_group_check=True,
            )
            return cps

        state = {"prev_carry": None, "out_lo": 0}

        def emit_stageC(U0, U1, sprinkle=None):
            for U in range(U0, U1):
                if sprinkle:
                    for off, fn in sprinkle:
                        if U == U0 + off:
                            fn()
                smax = min(S - 1, U)
                phi = (smax + 1) * B
                base = R * U

                for k in range(R):
                    q = base + k
                    if U == 0 and k == 0:
                        m_ap = big_m0[0:phi, :]
                    else:
                        mt_ = mpool.tile([P, W], F32, tag="m", name=f"m_{U}_{k}")
                        nc.vector.tensor_tensor(
                            mt_[0:phi, :],
                            tc_strip[0:phi, q - 1, 0:W],
                            tc_strip[0:phi, q - 1, 1:SLOT],
                            OP.min,
                        )
                        m_ap = mt_[0:phi, :]

                    if U == 0:
                        init = 0.0 if k == 0 else BIG
                    else:
                        init = state["prev_carry"][0:phi, k:k + 1]
                    nc.vector.tensor_tensor_scan(
                        tc_strip[0:phi, q, 1:SLOT],
                        m_ap,
                        tc_strip[0:phi, q, 1:SLOT],
                        init,
                        OP.min,
                        OP.add,
                    )
                    if U + 1 < T_TOT:
                        if k == R - 2:
                            state["cps"] = emit_carry(U, 0, R - 1)
                        elif k == R - 1:
                            cps = emit_carry(U, R - 1, R)
                            bnd = min(S - 1, U + 1) * B
                            nc.scalar.copy(
                                tc_strip[0:bnd + B, base + R:base + 2 * R, 0:1],
                                cps[0:bnd + B, 0:R])
                            state["prev_carry"] = state["cps"]
                # stream finished slots out every 16 supersteps
                if (U + 1) % 8 == 0 and U + 1 < T_TOT:
                    lo, hi = state["out_lo"], (U + 1) * R
                    nc.sync.dma_start(out=tc_out[:, lo:hi, :],
                                      in_=tc_strip[:, lo:hi, :])
                    state["out_lo"] = hi

        # Interleave stage-B blocks with stage-C chunks so each engine's
        # in-order queue pipelines across stages. C-chunk for block nt covers
        # supersteps [nt*PART/R, (nt+1)*PART/R).
        UPB = PART // R                 # supersteps per row-block
        for nt in range(1, NT):
            # spread block nt's batches across chunk nt-1's supersteps
            spr = [(min(2 * b_, UPB - 4), (lambda n_, bb: lambda: emit_stageB_batch(n_, bb))(nt, b_))
                   for b_ in range(B)]
            for qi in range(4):
                spr.append((UPB - 3 + min(qi, 2),
                            (lambda n_, q_: lambda: emit_hop2_part(n_, q_))(nt, qi)))
            emit_stageC((nt - 1) * UPB, nt * UPB, sprinkle=spr)
        emit_stageC((NT - 1) * UPB, T_TOT)

        # neg = logsumexp over m (emitted last; only needed at the end)
        negsum = neg_pool.tile([B, M], F32)
        nc.scalar.copy(negsum[:, :], ngb[:, :])
        mx = neg_pool.tile([B, 1], F32)
        nc.vector.reduce_max(mx[:], negsum[:], AX.X)
        sh = neg_pool.tile([B, M], F32)
        nc.vector.tensor_scalar(sh[:], negsum[:], mx[:], None, OP.subtract)
        ex = neg_pool.tile([B, M], F32)
        esum = neg_pool.tile([B, 1], F32)
        nc.scalar.activation(ex[:], sh[:], ACT.Exp, accum_out=esum[:])
        lg = neg_pool.tile([B, 1], F32)
        nc.scalar.activation(lg[:], esum[:], ACT.Ln)
        negv = neg_pool.tile([B, 1], F32)
        nc.vector.tensor_add(negv[:], lg[:], mx[:])
        nc.sync.dma_start(out=neg_out[:, :], in_=negv[:])

        lo = state["out_lo"]
        nc.sync.dma_start(out=tc_out[:, lo:SLOTS, :], in_=tc_strip[:, lo:SLOTS, :])

    nc.compile()
    return nc


# ---------------------------------------------------------------------------
# Host-side driver: sharding, run, unskew, backtrack walk, final loss
# ---------------------------------------------------------------------------
import numpy as np

B_TOT, N_G, M_G, D_G = 64, 512, 512, 256
N_CORES = 8
B_LOC = B_TOT // N_CORES
S_G, W_G, R_G = 16, 32, 8
P_G = S_G * B_LOC
SLOTS_G = N_G + R_G * S_G
SLOT_G = W_G + 1

_NC_CACHE = {}


def _get_nc():
    if "nc" not in _NC_CACHE:
        _NC_CACHE["nc"] = _build_cfg(B=B_LOC, N=N_G, M=M_G, D=D_G,
                                     S=S_G, W=W_G, R=R_G)
    return _NC_CACHE["nc"]


def _unskew(tc_skew):
    tc = np.empty((B_LOC, N_G, M_G), np.float32)
    for s in range(S_G):
        for b in range(B_LOC):
            tc[b, :, s * W_G:(s + 1) * W_G] = \
                tc_skew[s * B_LOC + b, R_G * s:R_G * s + N_G, 1:SLOT_G]
    return tc


def _host_finish(tc, x, y, neg):
    """Backtrack walk on the device tc + pos logsumexp (host side)."""
    Bt, Nn, Mm = tc.shape
    eps = 1e-8
    xn = x / np.maximum(np.linalg.norm(x, axis=-1, keepdims=True), eps)
    yn = y / np.maximum(np.linalg.norm(y, axis=-1, keepdims=True), eps)
    bidx = np.arange(Bt)
    i = np.full(Bt, Nn - 1, np.int64)
    j = np.full(Bt, Mm - 1, np.int64)
    Is, Js, Vs = [i.copy()], [j.copy()], [np.ones(Bt, bool)]
    active = (i > 0) & (j > 0)
    while active.any():
        a = tc[bidx, np.maximum(i - 1, 0), np.maximum(j - 1, 0)]
        bb = tc[bidx, np.maximum(i - 1, 0), j]
        c = tc[bidx, i, np.maximum(j - 1, 0)]
        diag = (a <= bb) & (a <= c)
        up = (~diag) & (bb <= c)
        ni = np.where(diag | up, i - 1, i)
        nj = np.where(diag | (~up), j - 1, j)
        i = np.where(active, ni, i)
        j = np.where(active, nj, j)
        Is.append(i.copy())
        Js.append(j.copy())
        Vs.append(active.copy())
        active = (i > 0) & (j > 0)
    at00 = (i == 0) & (j == 0)
    Is.append(np.zeros(Bt, np.int64))
    Js.append(np.zeros(Bt, np.int64))
    Vs.append(~at00)

    IS = np.stack(Is, 1)
    JS = np.stack(Js, 1)
    VS = np.stack(Vs, 1)
    costs = 1.0 - np.einsum("bld,bld->bl",
                            xn[bidx[:, None], IS], yn[bidx[:, None], JS])
    colsum = np.zeros((Bt, Mm), np.float32)
    np.add.at(colsum, (bidx[:, None], JS),
              np.where(VS, costs, 0.0).astype(np.float32))
    mxv = colsum.max(axis=1, keepdims=True)
    pos = (mxv + np.log(np.sum(np.exp(colsum - mxv),
                               axis=1, keepdims=True))).squeeze(1)
    return (pos.astype(np.float32) - neg).astype(np.float32)


def run_device(x, y, **kw):
    from concourse import bass_utils

    nc = _get_nc()
    in_maps = [
        {"x": np.ascontiguousarray(x[c * B_LOC:(c + 1) * B_LOC]),
         "y": np.ascontiguousarray(y[c * B_LOC:(c + 1) * B_LOC])}
        for c in range(N_CORES)
    ]
    res = bass_utils.run_bass_kernel_spmd(nc, in_maps, list(range(N_CORES)), **kw)
    tc = np.empty((B_TOT, N_G, M_G), np.float32)
    neg = np.empty(B_TOT, np.float32)
    for c in range(N_CORES):
        out = res.results[c]
        tc[c * B_LOC:(c + 1) * B_LOC] = _unskew(out["tc_out"])
        neg[c * B_LOC:(c + 1) * B_LOC] = out["neg_out"].reshape(B_LOC)
    return tc, neg, res


def kernel(x, y):
    x = np.asarray(x, dtype=np.float32)
    y = np.asarray(y, dtype=np.float32)
    tc, neg, _ = run_device(x, y)
    return _host_finish(tc, x, y, neg)



# revision 31
# speedup vs baseline: 1.0193x; 1.0193x over previous
"""nn_DTW kernel for 8 Trainium2 NeuronCores (batch data-parallel).

See _build_cfg for the device program; the host does the sequential
backtrack pointer-chase and the final logsumexp combine.

Structure:
  - stage A/B are pipelined in M-column "waves": y rows for wave mt are
    loaded, normalized (folded into the PE transpose via a diagonal
    moving matrix), transposed, and the block-0 cost columns for that
    wave are computed + hopped into the skewed wavefront layout, so the
    DTW wavefront starts after ~1 wave instead of after all loads.
  - stage C is a skew-R DTW wavefront: 16 strips x 8 batches in the 128
    partitions, tensor_tensor (min) + tensor_tensor_scan (min+add) per
    row-step on DVE; strip-to-strip carries go through a PE shift-matmul
    into PSUM, with the BIG boundary pre-set by Pool memsets.
  - x row-blocks 1..3 are loaded/prepped inside the wavefront
    (sprinkled), overlapping DMA + cost matmuls with the DVE chain.
"""

from contextlib import ExitStack

import concourse.bass as bass
import concourse.bacc as bacc
import concourse.tile as tile
from concourse import mybir
from concourse.masks import make_identity

F32 = mybir.dt.float32
AX = mybir.AxisListType
OP = mybir.AluOpType
ACT = mybir.ActivationFunctionType

BIG = 1.0e30


def _build_cfg(B=8, N=512, M=512, D=256, S=16, W=32, R=4, PART=128):
    assert S * W == M and N % R == 0
    P = S * B
    assert P <= PART
    NT = (N + PART - 1) // PART
    MT = (M + PART - 1) // PART
    DB = (D + PART - 1) // PART
    PN = min(PART, N)
    PD = min(PART, D)
    NSTEP = N // R
    T_TOT = NSTEP + S - 1
    SLOTS = N + R * S
    SLOT = W + 1
    SPW = S // MT                    # strips per column wave

    nc = bacc.Bacc("TRN2", target_bir_lowering=False, debug=False)

    x_in = nc.dram_tensor("x", [B, N, D], F32, kind="ExternalInput").ap()
    y_in = nc.dram_tensor("y", [B, M, D], F32, kind="ExternalInput").ap()
    tc_out = nc.dram_tensor("tc_out", [P, SLOTS, SLOT], F32, kind="ExternalOutput").ap()
    neg_out = nc.dram_tensor("neg_out", [B, 1], F32, kind="ExternalOutput").ap()
    cost_stage = nc.dram_tensor("cost_stage", [NT, B, PN, M], F32).ap()

    with tile.TileContext(nc) as tcx, ExitStack() as ctx:
        const = ctx.enter_context(tcx.tile_pool(name="const", bufs=1))
        ident = const.tile([PART, PART], F32)
        make_identity(nc, ident[:])
        oneh = const.tile([PN, B, B], F32)
        nc.vector.memset(oneh[:], 0.0)
        for b_ in range(B):
            nc.vector.memset(oneh[:, b_, b_:b_ + 1], 1.0)
        big_m0 = const.tile([P, W], F32)
        nc.vector.memset(big_m0[:], BIG)
        onecol = const.tile([PART, 1], F32)
        nc.vector.memset(onecol[:], 1.0)
        bigrow = const.tile([1, PART], F32)
        nc.vector.memset(bigrow[:], 0.0)
        nc.vector.memset(bigrow[0:1, 0:B], BIG)
        onesR = const.tile([1, R], F32)
        nc.vector.memset(onesR[:], 1.0)
        shift8 = const.tile([PART, PART], F32)
        nc.gpsimd.memset(shift8[:], 0.0)
        nc.gpsimd.affine_select(
            out=shift8[:], in_=shift8[:], compare_op=OP.not_equal, fill=1.0,
            base=B, pattern=[[-1, PART]], channel_multiplier=1,
        )

        strip = ctx.enter_context(tcx.tile_pool(name="strip", bufs=1))
        tc_strip = strip.tile([P, SLOTS, SLOT], F32)
        # Only the first R*S slots contain cells that are read before being
        # written (each strip's DP-row -1 boundary and strip 0's slot-0
        # column); everything later is written by hop2/scans first.
        nc.gpsimd.memset(tc_strip[:, 0:R * S, :], BIG)

        # persistent transposed operands + per-batch scales
        oper = ctx.enter_context(tcx.tile_pool(name="oper", bufs=1))
        xTall = oper.tile([PD, B, DB, N], F32)
        ynTall = oper.tile([PD, B, DB, M], F32)
        xrn_all = oper.tile([PN, B, NT], F32)

        stage = ctx.enter_context(tcx.tile_pool(name="stage", bufs=3))
        neg_pool = ctx.enter_context(tcx.tile_pool(name="negp", bufs=1))
        ps_c = ctx.enter_context(tcx.tile_pool(name="ps_c", bufs=1, space="PSUM"))
        ps_c0 = ctx.enter_context(tcx.tile_pool(name="ps_c0", bufs=2, space="PSUM"))
        ps_neg = ctx.enter_context(tcx.tile_pool(name="ps_neg", bufs=1, space="PSUM"))
        ngb = ps_neg.tile([B, M], F32, tag="ngb", bufs=1)

        xy = ctx.enter_context(tcx.tile_pool(name="xy", bufs=2))
        nrm = ctx.enter_context(tcx.tile_pool(name="nrm", bufs=3))
        xld = ctx.enter_context(tcx.tile_pool(name="xld", bufs=5))
        yld = ctx.enter_context(tcx.tile_pool(name="yld", bufs=9))
        dgp = ctx.enter_context(tcx.tile_pool(name="dgp", bufs=2))
        ps_t = ctx.enter_context(tcx.tile_pool(name="ps_t", bufs=2, space="PSUM"))

        # ---------------- stage A helpers ----------------
        def emit_xblock_load(nt, b):
            rows = min(PART, N - nt * PART)
            xt_t = xld.tile([PART, D], F32, tag="ldx", name=f"xb_{nt}_{b}")
            nc.sync.dma_start(out=xt_t[:rows, :],
                              in_=x_in[b, nt * PART:nt * PART + rows, :])
            return xt_t

        def _xprep_norm(nt, b, xt_t):
            rows = min(PART, N - nt * PART)
            sq = xy.tile([PART, D], F32, tag="sq")
            s2 = nrm.tile([PART, 1], F32, tag="s2")
            nc.scalar.activation(sq[:rows], xt_t[:rows, :], ACT.Square,
                                 accum_out=s2[:rows])
            nrm_t = nrm.tile([PART, 1], F32, tag="nrm")
            nc.scalar.activation(nrm_t[:rows], s2[:rows], ACT.Sqrt)
            rn = nrm.tile([PART, 1], F32, tag="rn")
            nc.vector.reciprocal(rn[:rows], nrm_t[:rows])
            nc.vector.tensor_scalar_mul(xrn_all[:rows, b, nt:nt + 1],
                                        rn[:rows], -1.0)

        def _xprep_transpose(nt, b, xt_t, db):
            rows = min(PART, N - nt * PART)
            dcols = min(PART, D - db * PART)
            pst = ps_t.tile([PD, PART], F32, tag="pstr",
                            name=f"psx{nt}_{b}_{db}")
            nc.tensor.matmul(pst[:dcols, :rows],
                             xt_t[:rows, db * PART:db * PART + dcols],
                             ident[:rows, :rows],
                             start=True, stop=True)
            nc.scalar.copy(xTall[:dcols, b, db, nt * PART:nt * PART + rows],
                           pst[:dcols, :rows])

        def emit_xblock_prep(nt, b, xt_t):
            _xprep_norm(nt, b, xt_t)
            for db in range(DB):
                _xprep_transpose(nt, b, xt_t, db)

        def emit_yblock(mt, b, yt_t):
            rows = min(PART, M - mt * PART)
            sq = xy.tile([PART, D], F32, tag="sq")
            s2 = nrm.tile([PART, 1], F32, tag="s2")
            nc.scalar.activation(sq[:rows], yt_t[:rows, :], ACT.Square,
                                 accum_out=s2[:rows])
            nrm_t = nrm.tile([PART, 1], F32, tag="nrm")
            nc.scalar.activation(nrm_t[:rows], s2[:rows], ACT.Sqrt)
            rn = nrm.tile([PART, 1], F32, tag="rn")
            nc.vector.reciprocal(rn[:rows], nrm_t[:rows])
            dg = dgp.tile([PART, PART], F32, tag="dg")
            nc.vector.tensor_scalar_mul(dg[:rows, :rows],
                                        ident[:rows, :rows], rn[:rows])
            for db in range(DB):
                dcols = min(PART, D - db * PART)
                pst = ps_t.tile([PD, PART], F32, tag="pstr",
                                name=f"psy{mt}_{b}_{db}")
                nc.tensor.matmul(pst[:dcols, :rows],
                                 yt_t[:rows, db * PART:db * PART + dcols],
                                 dg[:rows, :rows],
                                 start=True, stop=True)
                nc.scalar.copy(ynTall[:dcols, b, db, mt * PART:mt * PART + rows],
                               pst[:dcols, :rows])

        def emit_costB0_slice(b, mt):
            cols = min(PART, M - mt * PART)
            c0 = mt * PART
            psc = ps_c0.tile([PN, PART], F32, tag="psc0", name=f"psc0_{mt}_{b}")
            for db in range(DB):
                dcols = min(PART, D - db * PART)
                nc.tensor.matmul(
                    psc[:PN, :cols],
                    xTall[:dcols, b, db, 0:PN],
                    ynTall[:dcols, b, db, c0:c0 + cols],
                    start=(db == 0), stop=(db == DB - 1),
                )
            cn = stage.tile([PN, PART], F32, tag="cn0", name=f"cn0_{mt}_{b}")
            nc.scalar.activation(cn[:PN, :cols], psc[:PN, :cols], ACT.Copy,
                                 scale=xrn_all[:PN, b, 0:1], bias=1.0)
            nc.tensor.matmul(
                ngb[:, c0:c0 + cols],
                oneh[:PN, b, :],
                cn[:PN, :cols],
                start=(b == 0), stop=False,
                skip_group_check=True,
            )
            heng = nc.scalar if b % 2 == 0 else nc.sync
            heng.dma_start(out=cost_stage[0, b, :, c0:c0 + cols],
                           in_=cn[:PN, :cols])

        # full-width cost for row blocks nt >= 1 (sprinkled into wavefront)
        def _stageB_mm(nt, b, db, psc):
            rows = min(PART, N - nt * PART)
            dcols = min(PART, D - db * PART)
            nc.tensor.matmul(
                psc[:rows, :],
                xTall[:dcols, b, db, nt * PART:nt * PART + rows],
                ynTall[:dcols, b, db, :],
                start=(db == 0), stop=(db == DB - 1),
            )

        def _stageB_fin(nt, b, psc):
            rows = min(PART, N - nt * PART)
            cn = stage.tile([PN, M], F32, tag="cn", name=f"cn_{nt}_{b}")
            nc.scalar.activation(cn[:rows], psc[:rows], ACT.Copy,
                                 scale=xrn_all[:rows, b, nt:nt + 1], bias=1.0)
            nc.tensor.matmul(
                ngb[:, :],
                oneh[:rows, b, :],
                cn[:rows, :],
                start=False,
                stop=(nt == NT - 1 and b == B - 1),
                skip_group_check=True,
            )
            heng = nc.scalar if b % 2 == 0 else nc.sync
            heng.dma_start(out=cost_stage[nt, b], in_=cn[:rows, :])

        def emit_stageB_batch(nt, b):
            psc = ps_c.tile([PN, M], F32, tag="psc", name=f"psc_{nt}_{b}")
            for db in range(DB):
                _stageB_mm(nt, b, db, psc)
            _stageB_fin(nt, b, psc)

        def emit_hop2_part(nt, quarter):
            rows = min(PART, N - nt * PART)
            for s in range(quarter * SPW, (quarter + 1) * SPW):
                src = cost_stage[nt, :, :, s * W:(s + 1) * W]
                eng = nc.sync if s % 2 == 0 else nc.scalar
                eng.dma_start(
                    out=tc_strip[s * B:s * B + B,
                                 R * s + nt * PART:R * s + nt * PART + rows,
                                 1:SLOT],
                    in_=src)

        # ---------------- stage A/B0: column-wave pipeline ----------------
        def emit_yload(mt, b):
            rows = min(PART, M - mt * PART)
            yt_t = yld.tile([PART, D], F32, tag="ldy", name=f"yb_{mt}_{b}")
            nc.sync.dma_start(out=yt_t[:rows, :],
                              in_=y_in[b, mt * PART:mt * PART + rows, :])
            return yt_t

        # wave 0 + x block 0 pre-emitted; waves 1..MT-1 are sprinkled into
        # the first wavefront chunk so early carries aren't queued behind them
        for b in range(B):
            xt_t = emit_xblock_load(0, b)
            emit_xblock_prep(0, b, xt_t)
        y0ts = [emit_yload(0, b) for b in range(B)]
        for b in range(B):
            emit_yblock(0, b, y0ts[b])
        for b in range(B):
            emit_costB0_slice(b, 0)
        emit_hop2_part(0, 0)

        # ---------------- Stage C: skew-R DTW wavefront ----------------
        ps_carry = ctx.enter_context(tcx.tile_pool(name="ps_cr", bufs=1, space="PSUM"))
        mpool = ctx.enter_context(tcx.tile_pool(name="mpool", bufs=8))

        NCARRY = 2
        carry_tiles = [
            ps_carry.tile([P, R], F32, tag=f"cr{i}", name=f"carry{i}")
            for i in range(NCARRY)
        ]

        def emit_carry_pre(U):
            # boundary init: BIG for strip 0, 0 elsewhere (constant operands,
            # so this PE matmul has no scan deps); the shift matmuls then
            # accumulate (start=False) on top
            bnd = min(S - 1, U + 1) * B
            cps = carry_tiles[(U + 1) % NCARRY]
            nc.tensor.matmul(
                cps[0:bnd + B, 0:R],
                bigrow[0:1, 0:bnd + B],
                onesR[0:1, 0:R],
                start=True, stop=False, skip_group_check=True,
            )
            return cps

        def emit_carry(U, c0, c1):
            base = R * U
            bnd = min(S - 1, U + 1) * B
            cps = carry_tiles[(U + 1) % NCARRY]
            nc.tensor.matmul(
                cps[0:bnd + B, c0:c1],
                shift8[0:bnd, 0:bnd + B],
                tc_strip[0:bnd, base + c0:base + c1, SLOT - 1:SLOT],
                start=False, stop=True, skip_group_check=True,
            )
            return cps

        state = {"prev_carry": None, "out_lo": 0}

        def emit_stageC(U0, U1, sprinkle=None):
            for U in range(U0, U1):
                if sprinkle:
                    for off, fn in sprinkle:
                        if U == U0 + off:
                            fn()
                smax = min(S - 1, U)
                phi = (smax + 1) * B
                base = R * U

                for k in range(R):
                    q = base + k
                    if U == 0 and k == 0:
                        m_ap = big_m0[0:phi, :]
                    else:
                        mt_ = mpool.tile([P, W], F32, tag="m", name=f"m_{U}_{k}")
                        nc.vector.tensor_tensor(
                            mt_[0:phi, :],
                            tc_strip[0:phi, q - 1, 0:W],
                            tc_strip[0:phi, q - 1, 1:SLOT],
                            OP.min,
                        )
                        m_ap = mt_[0:phi, :]

                    if U == 0:
                        init = 0.0 if k == 0 else BIG
                    else:
                        init = state["prev_carry"][0:phi, k:k + 1]
                    nc.vector.tensor_tensor_scan(
                        tc_strip[0:phi, q, 1:SLOT],
                        m_ap,
                        tc_strip[0:phi, q, 1:SLOT],
                        init,
                        OP.min,
                        OP.add,
                    )
                    if U + 1 < T_TOT:
                        if k == 0:
                            emit_carry_pre(U)
                        if k == max(1, R // 2 - 1):
                            cps = emit_carry(U, 0, k + 1)
                            state["cps"] = cps
                            bnd = min(S - 1, U + 1) * B
                            nc.scalar.copy(
                                tc_strip[0:bnd + B, base + R:base + R + k + 1, 0:1],
                                cps[0:bnd + B, 0:k + 1])
                        elif k == R - 1:
                            c0 = max(2, R // 2)
                            cps = emit_carry(U, c0, R)
                            bnd = min(S - 1, U + 1) * B
                            nc.scalar.copy(
                                tc_strip[0:bnd + B, base + R + c0:base + 2 * R, 0:1],
                                cps[0:bnd + B, c0:R])
                            state["prev_carry"] = state["cps"]
                # stream finished slots out periodically
                if (U + 1) % 8 == 0 and U + 1 < T_TOT:
                    lo, hi = state["out_lo"], (U + 1) * R
                    nc.sync.dma_start(out=tc_out[:, lo:hi, :],
                                      in_=tc_strip[:, lo:hi, :])
                    state["out_lo"] = hi

        # Interleave x blocks 1..NT-1 (load, prep, cost, hop) with stage-C
        # chunks so each engine's in-order queue pipelines across stages.
        UPB = PART // R                 # supersteps per row-block
        xtile_box = {}

        def mk_xload(n_, bb):
            def f():
                xtile_box[(n_, bb)] = emit_xblock_load(n_, bb)
            return f

        def mk_xprep(n_, bb):
            def f():
                emit_xblock_prep(n_, bb, xtile_box.pop((n_, bb)))
            return f

        def mk_xcost(n_, bb):
            def f():
                emit_stageB_batch(n_, bb)
            return f

        ytile_box = {}

        def mk_yload(mt, bs):
            def f():
                for b in bs:
                    ytile_box[(mt, b)] = emit_yload(mt, b)
            return f

        def mk_wave_pc(mt, bs):
            def f():
                for b in bs:
                    emit_yblock(mt, b, ytile_box.pop((mt, b)))
                for b in bs:
                    emit_costB0_slice(b, mt)
            return f

        WQ = max(1, UPB // MT)          # superstep span per sprinkled wave
        for nt in range(1, NT):
            spr = []
            if nt == 1:
                for mt_ in range(1, MT):
                    o0 = WQ * (mt_ - 1)
                    spr.append((o0, mk_yload(mt_, list(range(B)))))
                    for oi in range(4):
                        spr.append((o0 + oi * WQ // 4,
                                    mk_wave_pc(mt_, [2 * oi, 2 * oi + 1])))
                    spr.append((o0 + WQ, (lambda m_: lambda: emit_hop2_part(0, m_))(mt_)))
                xl0, xp0, xc0 = UPB - 10, UPB - 6, UPB - 5
            else:
                xl0, xp0, xc0 = 0, 2, 3
            for b_ in range(B):
                spr.append((min(xl0 + b_ // 2, UPB - 6), mk_xload(nt, b_)))
                spr.append((min(xp0 + b_ // 2, UPB - 3), mk_xprep(nt, b_)))
                spr.append((min(xc0 + b_ // 2, UPB - 2), mk_xcost(nt, b_)))
            for qi in range(MT):
                spr.append((UPB - 2 + min(qi, 1),
                            (lambda n_, q_: lambda: emit_hop2_part(n_, q_))(nt, qi)))
            spr.sort(key=lambda e: e[0])
            emit_stageC((nt - 1) * UPB, nt * UPB, sprinkle=spr)
        emit_stageC((NT - 1) * UPB, T_TOT)

        # neg = logsumexp over m (emitted last; only needed at the end)
        negsum = neg_pool.tile([B, M], F32)
        nc.scalar.copy(negsum[:, :], ngb[:, :])
        mx = neg_pool.tile([B, 1], F32)
        nc.vector.reduce_max(mx[:], negsum[:], AX.X)
        sh = neg_pool.tile([B, M], F32)
        nc.vector.tensor_scalar(sh[:], negsum[:], mx[:], None, OP.subtract)
        ex = neg_pool.tile([B, M], F32)
        esum = neg_pool.tile([B, 1], F32)
        nc.scalar.activation(ex[:], sh[:], ACT.Exp, accum_out=esum[:])
        lg = neg_pool.tile([B, 1], F32)
        nc.scalar.activation(lg[:], esum[:], ACT.Ln)
        negv = neg_pool.tile([B, 1], F32)
        nc.vector.tensor_add(negv[:], lg[:], mx[:])
        nc.sync.dma_start(out=neg_out[:, :], in_=negv[:])

        lo = state["out_lo"]
        nc.sync.dma_start(out=tc_out[:, lo:SLOTS, :], in_=tc_strip[:, lo:SLOTS, :])

    nc.compile()
    return nc


# ---------------------------------------------------------------------------
# Host-side driver: sharding, run, unskew, backtrack walk, final loss
# ---------------------------------------------------------------------------
import numpy as np

B_TOT, N_G, M_G, D_G = 64, 512, 512, 256
N_CORES = 8
B_LOC = B_TOT // N_CORES
S_G, W_G, R_G = 16, 32, 8
P_G = S_G * B_LOC
SLOTS_G = N_G + R_G * S_G
SLOT_G = W_G + 1

_NC_CACHE = {}


def _get_nc():
    if "nc" not in _NC_CACHE:
        _NC_CACHE["nc"] = _build_cfg(B=B_LOC, N=N_G, M=M_G, D=D_G,
                                     S=S_G, W=W_G, R=R_G)
    return _NC_CACHE["nc"]


def _unskew(tc_skew):
    tc = np.empty((B_LOC, N_G, M_G), np.float32)
    for s in range(S_G):
        for b in range(B_LOC):
            tc[b, :, s * W_G:(s + 1) * W_G] = \
                tc_skew[s * B_LOC + b, R_G * s:R_G * s + N_G, 1:SLOT_G]
    return tc


def _host_finish(tc, x, y, neg):
    """Backtrack walk on the device tc + pos logsumexp (host side)."""
    Bt, Nn, Mm = tc.shape
    eps = 1e-8
    xn = x / np.maximum(np.linalg.norm(x, axis=-1, keepdims=True), eps)
    yn = y / np.maximum(np.linalg.norm(y, axis=-1, keepdims=True), eps)
    bidx = np.arange(Bt)
    i = np.full(Bt, Nn - 1, np.int64)
    j = np.full(Bt, Mm - 1, np.int64)
    Is, Js, Vs = [i.copy()], [j.copy()], [np.ones(Bt, bool)]
    active = (i > 0) & (j > 0)
    while active.any():
        a = tc[bidx, np.maximum(i - 1, 0), np.maximum(j - 1, 0)]
        bb = tc[bidx, np.maximum(i - 1, 0), j]
        c = tc[bidx, i, np.maximum(j - 1, 0)]
        diag = (a <= bb) & (a <= c)
        up = (~diag) & (bb <= c)
        ni = np.where(diag | up, i - 1, i)
        nj = np.where(diag | (~up), j - 1, j)
        i = np.where(active, ni, i)
        j = np.where(active, nj, j)
        Is.append(i.copy())
        Js.append(j.copy())
        Vs.append(active.copy())
        active = (i > 0) & (j > 0)
    at00 = (i == 0) & (j == 0)
    Is.append(np.zeros(Bt, np.int64))
    Js.append(np.zeros(Bt, np.int64))
    Vs.append(~at00)

    IS = np.stack(Is, 1)
    JS = np.stack(Js, 1)
    VS = np.stack(Vs, 1)
    costs = 1.0 - np.einsum("bld,bld->bl",
                            xn[bidx[:, None], IS], yn[bidx[:, None], JS])
    colsum = np.zeros((Bt, Mm), np.float32)
    np.add.at(colsum, (bidx[:, None], JS),
              np.where(VS, costs, 0.0).astype(np.float32))
    mxv = colsum.max(axis=1, keepdims=True)
    pos = (mxv + np.log(np.sum(np.exp(colsum - mxv),
                               axis=1, keepdims=True))).squeeze(1)
    return (pos.astype(np.float32) - neg).astype(np.float32)


def run_device(x, y, **kw):
    from concourse import bass_utils

    nc = _get_nc()
    in_maps = [
        {"x": np.ascontiguousarray(x[c * B_LOC:(c + 1) * B_LOC]),
         "y": np.ascontiguousarray(y[c * B_LOC:(c + 1) * B_LOC])}
        for c in range(N_CORES)
    ]
    res = bass_utils.run_bass_kernel_spmd(nc, in_maps, list(range(N_CORES)), **kw)
    tc = np.empty((B_TOT, N_G, M_G), np.float32)
    neg = np.empty(B_TOT, np.float32)
    for c in range(N_CORES):
        out = res.results[c]
        tc[c * B_LOC:(c + 1) * B_LOC] = _unskew(out["tc_out"])
        neg[c * B_LOC:(c + 1) * B_LOC] = out["neg_out"].reshape(B_LOC)
    return tc, neg, res


def kernel(x, y):
    x = np.asarray(x, dtype=np.float32)
    y = np.asarray(y, dtype=np.float32)
    tc, neg, _ = run_device(x, y)
    return _host_finish(tc, x, y, neg)


# revision 41
# speedup vs baseline: 1.0549x; 1.0350x over previous
"""nn_DTW kernel for 8 Trainium2 NeuronCores (batch data-parallel).

See _build_cfg for the device program; the host does the sequential
backtrack pointer-chase and the final logsumexp combine.

Structure:
  - stage A/B are pipelined in M-column "waves": y rows for wave mt are
    loaded, normalized (folded into the PE transpose via a diagonal
    moving matrix), transposed, and the block-0 cost columns for that
    wave are computed + hopped into the skewed wavefront layout, so the
    DTW wavefront starts after ~1 wave instead of after all loads.
  - stage C is a skew-R DTW wavefront: 16 strips x 8 batches in the 128
    partitions, tensor_tensor (min) + tensor_tensor_scan (min+add) per
    row-step on DVE; strip-to-strip carries go through a PE shift-matmul
    into PSUM, with the BIG boundary pre-set by Pool memsets.
  - x row-blocks 1..3 are loaded/prepped inside the wavefront
    (sprinkled), overlapping DMA + cost matmuls with the DVE chain.
"""

from contextlib import ExitStack

import concourse.bass as bass
import concourse.bacc as bacc
import concourse.tile as tile
from concourse import mybir
from concourse.masks import make_identity

F32 = mybir.dt.float32
AX = mybir.AxisListType
OP = mybir.AluOpType
ACT = mybir.ActivationFunctionType

BIG = 1.0e30


def _build_cfg(B=8, N=512, M=512, D=256, S=16, W=32, R=4, PART=128):
    assert S * W == M and N % R == 0
    P = S * B
    assert P <= PART
    NT = (N + PART - 1) // PART
    MT = (M + PART - 1) // PART
    DB = (D + PART - 1) // PART
    PN = min(PART, N)
    PD = min(PART, D)
    NSTEP = N // R
    T_TOT = NSTEP + S - 1
    SLOTS = N + R * S
    SLOT = W + 1
    SPW = S // MT                    # strips per column wave

    nc = bacc.Bacc("TRN2", target_bir_lowering=False, debug=False)

    x_in = nc.dram_tensor("x", [B, N, D], F32, kind="ExternalInput").ap()
    y_in = nc.dram_tensor("y", [B, M, D], F32, kind="ExternalInput").ap()
    tc_out = nc.dram_tensor("tc_out", [P, SLOTS, SLOT], F32, kind="ExternalOutput").ap()
    neg_out = nc.dram_tensor("neg_out", [B, 1], F32, kind="ExternalOutput").ap()
    cost_stage = nc.dram_tensor("cost_stage", [NT, B, PN, M], F32).ap()

    with tile.TileContext(nc) as tcx, ExitStack() as ctx:
        const = ctx.enter_context(tcx.tile_pool(name="const", bufs=1))
        ident = const.tile([PART, PART], F32)
        make_identity(nc, ident[:])
        oneh = const.tile([PN, B, B], F32)
        nc.vector.memset(oneh[:], 0.0)
        for b_ in range(B):
            nc.vector.memset(oneh[:, b_, b_:b_ + 1], 1.0)
        big_m0 = const.tile([P, W], F32)
        nc.vector.memset(big_m0[:], BIG)
        onecol = const.tile([PART, 1], F32)
        nc.vector.memset(onecol[:], 1.0)
        bigrow = const.tile([1, PART], F32)
        nc.vector.memset(bigrow[:], 0.0)
        nc.vector.memset(bigrow[0:1, 0:B], BIG)
        onesR = const.tile([1, R], F32)
        nc.vector.memset(onesR[:], 1.0)
        shift8 = const.tile([PART, PART], F32)
        nc.gpsimd.memset(shift8[:], 0.0)
        nc.gpsimd.affine_select(
            out=shift8[:], in_=shift8[:], compare_op=OP.not_equal, fill=1.0,
            base=B, pattern=[[-1, PART]], channel_multiplier=1,
        )

        strip = ctx.enter_context(tcx.tile_pool(name="strip", bufs=1))
        tc_strip = strip.tile([P, SLOTS, SLOT], F32)
        # Only the first R*S slots contain cells that are read before being
        # written (each strip's DP-row -1 boundary and strip 0's slot-0
        # column); everything later is written by hop2/scans first.
        nc.gpsimd.memset(tc_strip[:, 0:R * S, :], BIG)

        # persistent transposed operands + per-batch scales
        oper = ctx.enter_context(tcx.tile_pool(name="oper", bufs=1))
        xTall = oper.tile([PD, B, DB, N], F32)
        ynTall = oper.tile([PD, B, DB, M], F32)
        xrn_all = oper.tile([PN, B, NT], F32)

        stage = ctx.enter_context(tcx.tile_pool(name="stage", bufs=3))
        neg_pool = ctx.enter_context(tcx.tile_pool(name="negp", bufs=1))
        ps_c = ctx.enter_context(tcx.tile_pool(name="ps_c", bufs=2, space="PSUM"))
        ps_c0 = ctx.enter_context(tcx.tile_pool(name="ps_c0", bufs=1, space="PSUM"))
        ps_neg = ctx.enter_context(tcx.tile_pool(name="ps_neg", bufs=1, space="PSUM"))
        ngb = ps_neg.tile([B, M], F32, tag="ngb", bufs=1)

        xy = ctx.enter_context(tcx.tile_pool(name="xy", bufs=2))
        nrm = ctx.enter_context(tcx.tile_pool(name="nrm", bufs=3))
        xld = ctx.enter_context(tcx.tile_pool(name="xld", bufs=5))
        yld = ctx.enter_context(tcx.tile_pool(name="yld", bufs=9))
        dgp = ctx.enter_context(tcx.tile_pool(name="dgp", bufs=3))
        ps_t = ctx.enter_context(tcx.tile_pool(name="ps_t", bufs=2, space="PSUM"))

        # ---------------- stage A helpers ----------------
        def emit_xblock_load(nt, b):
            rows = min(PART, N - nt * PART)
            xt_t = xld.tile([PART, D], F32, tag="ldx", name=f"xb_{nt}_{b}")
            nc.sync.dma_start(out=xt_t[:rows, :],
                              in_=x_in[b, nt * PART:nt * PART + rows, :])
            return xt_t

        def _xprep_norm(nt, b, xt_t):
            rows = min(PART, N - nt * PART)
            sq = xy.tile([PART, D], F32, tag="sq")
            s2 = nrm.tile([PART, 1], F32, tag="s2")
            nc.scalar.activation(sq[:rows], xt_t[:rows, :], ACT.Square,
                                 accum_out=s2[:rows])
            nrm_t = nrm.tile([PART, 1], F32, tag="nrm")
            nc.scalar.activation(nrm_t[:rows], s2[:rows], ACT.Sqrt)
            rn = nrm.tile([PART, 1], F32, tag="rn")
            nc.vector.reciprocal(rn[:rows], nrm_t[:rows])
            nc.vector.tensor_scalar_mul(xrn_all[:rows, b, nt:nt + 1],
                                        rn[:rows], -1.0)

        def _xprep_transpose(nt, b, xt_t, db):
            rows = min(PART, N - nt * PART)
            dcols = min(PART, D - db * PART)
            pst = ps_t.tile([PD, PART], F32, tag="pstr",
                            name=f"psx{nt}_{b}_{db}")
            nc.tensor.matmul(pst[:dcols, :rows],
                             xt_t[:rows, db * PART:db * PART + dcols],
                             ident[:rows, :rows],
                             start=True, stop=True)
            nc.scalar.copy(xTall[:dcols, b, db, nt * PART:nt * PART + rows],
                           pst[:dcols, :rows])

        def emit_xblock_prep(nt, b, xt_t):
            _xprep_norm(nt, b, xt_t)
            for db in range(DB):
                _xprep_transpose(nt, b, xt_t, db)

        def emit_yblock(mt, b, yt_t):
            rows = min(PART, M - mt * PART)
            sq = xy.tile([PART, D], F32, tag="sq")
            s2 = nrm.tile([PART, 1], F32, tag="s2")
            nc.scalar.activation(sq[:rows], yt_t[:rows, :], ACT.Square,
                                 accum_out=s2[:rows])
            nrm_t = nrm.tile([PART, 1], F32, tag="nrm")
            nc.scalar.activation(nrm_t[:rows], s2[:rows], ACT.Sqrt)
            rn = nrm.tile([PART, 1], F32, tag="rn")
            nc.vector.reciprocal(rn[:rows], nrm_t[:rows])
            dg = dgp.tile([PART, PART], F32, tag="dg")
            nc.vector.tensor_scalar_mul(dg[:rows, :rows],
                                        ident[:rows, :rows], rn[:rows])
            for db in range(DB):
                dcols = min(PART, D - db * PART)
                pst = ps_t.tile([PD, PART], F32, tag="pstr",
                                name=f"psy{mt}_{b}_{db}")
                nc.tensor.matmul(pst[:dcols, :rows],
                                 yt_t[:rows, db * PART:db * PART + dcols],
                                 dg[:rows, :rows],
                                 start=True, stop=True)
                nc.scalar.copy(ynTall[:dcols, b, db, mt * PART:mt * PART + rows],
                               pst[:dcols, :rows])

        def emit_costB0_slice(b, mt):
            cols = min(PART, M - mt * PART)
            c0 = mt * PART
            psc = ps_c0.tile([PN, PART], F32, tag="psc0", name=f"psc0_{mt}_{b}")
            for db in range(DB):
                dcols = min(PART, D - db * PART)
                nc.tensor.matmul(
                    psc[:PN, :cols],
                    xTall[:dcols, b, db, 0:PN],
                    ynTall[:dcols, b, db, c0:c0 + cols],
                    start=(db == 0), stop=(db == DB - 1),
                )
            cn = stage.tile([PN, PART], F32, tag="cn0", name=f"cn0_{mt}_{b}")
            nc.scalar.activation(cn[:PN, :cols], psc[:PN, :cols], ACT.Copy,
                                 scale=xrn_all[:PN, b, 0:1], bias=1.0)
            nc.tensor.matmul(
                ngb[:, c0:c0 + cols],
                oneh[:PN, b, :],
                cn[:PN, :cols],
                start=(b == 0), stop=False,
                skip_group_check=True,
            )
            heng = nc.sync
            heng.dma_start(out=cost_stage[0, b, :, c0:c0 + cols],
                           in_=cn[:PN, :cols])

        # full-width cost for row blocks nt >= 1 (sprinkled into wavefront)
        def _stageB_mm(nt, b, db, psc):
            rows = min(PART, N - nt * PART)
            dcols = min(PART, D - db * PART)
            nc.tensor.matmul(
                psc[:rows, :],
                xTall[:dcols, b, db, nt * PART:nt * PART + rows],
                ynTall[:dcols, b, db, :],
                start=(db == 0), stop=(db == DB - 1),
            )

        def _stageB_fin(nt, b, psc):
            rows = min(PART, N - nt * PART)
            cn = stage.tile([PN, M], F32, tag="cn", name=f"cn_{nt}_{b}")
            nc.scalar.activation(cn[:rows], psc[:rows], ACT.Copy,
                                 scale=xrn_all[:rows, b, nt:nt + 1], bias=1.0)
            nc.tensor.matmul(
                ngb[:, :],
                oneh[:rows, b, :],
                cn[:rows, :],
                start=False,
                stop=(nt == NT - 1 and b == B - 1),
                skip_group_check=True,
            )
            nc.sync.dma_start(out=cost_stage[nt, b], in_=cn[:rows, :])

        def emit_stageB_batch(nt, b):
            psc = ps_c.tile([PN, M], F32, tag="psc", name=f"psc_{nt}_{b}")
            for db in range(DB):
                _stageB_mm(nt, b, db, psc)
            _stageB_fin(nt, b, psc)

        def emit_hop2_part(nt, quarter):
            rows = min(PART, N - nt * PART)
            for s in range(quarter * SPW, (quarter + 1) * SPW):
                src = cost_stage[nt, :, :, s * W:(s + 1) * W]
                eng = nc.sync if s % 2 == 0 else nc.scalar
                eng.dma_start(
                    out=tc_strip[s * B:s * B + B,
                                 R * s + nt * PART:R * s + nt * PART + rows,
                                 1:SLOT],
                    in_=src)

        # ---------------- stage A/B0: column-wave pipeline ----------------
        def emit_yload(mt, b):
            rows = min(PART, M - mt * PART)
            yt_t = yld.tile([PART, D], F32, tag="ldy", name=f"yb_{mt}_{b}")
            nc.sync.dma_start(out=yt_t[:rows, :],
                              in_=y_in[b, mt * PART:mt * PART + rows, :])
            return yt_t

        # wave 0 + x block 0 pre-emitted; waves 1..MT-1 are sprinkled into
        # the first wavefront chunk so early carries aren't queued behind them
        for b in range(B):
            xt_t = emit_xblock_load(0, b)
            emit_xblock_prep(0, b, xt_t)
        y0ts = [emit_yload(0, b) for b in range(B)]
        for b in range(B):
            emit_yblock(0, b, y0ts[b])
        for b in range(B):
            emit_costB0_slice(b, 0)
        emit_hop2_part(0, 0)

        # ---------------- Stage C: skew-R DTW wavefront ----------------
        ps_carry = ctx.enter_context(tcx.tile_pool(name="ps_cr", bufs=1, space="PSUM"))
        mpool = ctx.enter_context(tcx.tile_pool(name="mpool", bufs=12))

        NCARRY = 2
        carry_tiles = [
            ps_carry.tile([P, R], F32, tag=f"cr{i}", name=f"carry{i}")
            for i in range(NCARRY)
        ]

        def emit_carry_pre(U):
            # boundary init: BIG for strip 0, 0 elsewhere (constant operands,
            # so this PE matmul has no scan deps); the shift matmuls then
            # accumulate (start=False) on top
            bnd = min(S - 1, U + 1) * B
            cps = carry_tiles[(U + 1) % NCARRY]
            nc.tensor.matmul(
                cps[0:bnd + B, 0:R],
                bigrow[0:1, 0:bnd + B],
                onesR[0:1, 0:R],
                start=True, stop=False, skip_group_check=True,
            )
            return cps

        def emit_carry(U, c0, c1):
            base = R * U
            bnd = min(S - 1, U + 1) * B
            cps = carry_tiles[(U + 1) % NCARRY]
            nc.tensor.matmul(
                cps[0:bnd + B, c0:c1],
                shift8[0:bnd, 0:bnd + B],
                tc_strip[0:bnd, base + c0:base + c1, SLOT - 1:SLOT],
                start=False, stop=True, skip_group_check=True,
            )
            return cps

        state = {"prev_carry": None, "out_lo": 0}

        def emit_stageC(U0, U1, sprinkle=None):
            for U in range(U0, U1):
                if sprinkle:
                    for off, fn in sprinkle:
                        if U == U0 + off:
                            fn()
                smax = min(S - 1, U)
                phi = (smax + 1) * B
                base = R * U

                for k in range(R):
                    q = base + k
                    if U == 0 and k == 0:
                        m_ap = big_m0[0:phi, :]
                    else:
                        mt_ = mpool.tile([P, W], F32, tag="m", name=f"m_{U}_{k}")
                        nc.vector.tensor_tensor(
                            mt_[0:phi, :],
                            tc_strip[0:phi, q - 1, 0:W],
                            tc_strip[0:phi, q - 1, 1:SLOT],
                            OP.min,
                        )
                        m_ap = mt_[0:phi, :]

                    if U == 0:
                        init = 0.0 if k == 0 else BIG
                    else:
                        init = state["prev_carry"][0:phi, k:k + 1]
                    nc.vector.tensor_tensor_scan(
                        tc_strip[0:phi, q, 1:SLOT],
                        m_ap,
                        tc_strip[0:phi, q, 1:SLOT],
                        init,
                        OP.min,
                        OP.add,
                    )
                    if U + 1 < T_TOT:
                        if k == 0:
                            emit_carry_pre(U)
                        if k == max(1, R // 2 - 1):
                            state["cps"] = emit_carry(U, 0, k + 1)
                        elif k == R - 1:
                            cps = emit_carry(U, max(2, R // 2), R)
                            bnd = min(S - 1, U + 1) * B
                            nc.scalar.copy(
                                tc_strip[0:bnd + B, base + R:base + 2 * R, 0:1],
                                cps[0:bnd + B, 0:R])
                            state["prev_carry"] = state["cps"]
                # stream finished slots out periodically
                if (U + 1) % 8 == 0 and U + 1 < T_TOT:
                    lo, hi = state["out_lo"], (U + 1) * R
                    nc.sync.dma_start(out=tc_out[:, lo:hi, :],
                                      in_=tc_strip[:, lo:hi, :])
                    state["out_lo"] = hi

        # Interleave x blocks 1..NT-1 (load, prep, cost, hop) with stage-C
        # chunks so each engine's in-order queue pipelines across stages.
        UPB = PART // R                 # supersteps per row-block
        xtile_box = {}

        def mk_xload(n_, bb):
            def f():
                xtile_box[(n_, bb)] = emit_xblock_load(n_, bb)
            return f

        def mk_xprep(n_, bb):
            def f():
                emit_xblock_prep(n_, bb, xtile_box.pop((n_, bb)))
            return f

        def mk_xcost(n_, bb):
            def f():
                emit_stageB_batch(n_, bb)
            return f

        ytile_box = {}

        def mk_yload(mt, bs):
            def f():
                for b in bs:
                    ytile_box[(mt, b)] = emit_yload(mt, b)
            return f

        def mk_wave_pc(mt, bs):
            def f():
                for b in bs:
                    emit_yblock(mt, b, ytile_box.pop((mt, b)))
                for b in bs:
                    emit_costB0_slice(b, mt)
            return f

        WQ = max(1, (UPB - 4) // MT)    # superstep span per sprinkled wave
        for nt in range(1, NT):
            spr = []
            if nt == 1:
                for mt_ in range(1, MT):
                    o0 = WQ * (mt_ - 1)
                    spr.append((o0, mk_yload(mt_, list(range(B)))))
                    for oi in range(4):
                        spr.append((o0 + oi * WQ // 4,
                                    mk_wave_pc(mt_, [2 * oi, 2 * oi + 1])))
                    spr.append((o0 + WQ, (lambda m_: lambda: emit_hop2_part(0, m_))(mt_)))
                xl0, xp0, xc0 = UPB - 10, UPB - 6, UPB - 5
            else:
                xl0, xp0, xc0 = 0, 2, 3
            for b_ in range(B):
                spr.append((min(xl0 + b_ // 2, UPB - 6), mk_xload(nt, b_)))
                spr.append((min(xp0 + b_ // 2, UPB - 3), mk_xprep(nt, b_)))
                spr.append((min(xc0 + b_ // 2, UPB - 2), mk_xcost(nt, b_)))
            for qi in range(MT):
                spr.append((UPB - 2 + min(qi, 1),
                            (lambda n_, q_: lambda: emit_hop2_part(n_, q_))(nt, qi)))
            spr.sort(key=lambda e: e[0])
            emit_stageC((nt - 1) * UPB, nt * UPB, sprinkle=spr)
        emit_stageC((NT - 1) * UPB, T_TOT)

        # neg = logsumexp over m (emitted last; only needed at the end)
        negsum = neg_pool.tile([B, M], F32)
        nc.scalar.copy(negsum[:, :], ngb[:, :])
        mx = neg_pool.tile([B, 1], F32)
        nc.vector.reduce_max(mx[:], negsum[:], AX.X)
        sh = neg_pool.tile([B, M], F32)
        nc.vector.tensor_scalar(sh[:], negsum[:], mx[:], None, OP.subtract)
        ex = neg_pool.tile([B, M], F32)
        esum = neg_pool.tile([B, 1], F32)
        nc.scalar.activation(ex[:], sh[:], ACT.Exp, accum_out=esum[:])
        lg = neg_pool.tile([B, 1], F32)
        nc.scalar.activation(lg[:], esum[:], ACT.Ln)
        negv = neg_pool.tile([B, 1], F32)
        nc.vector.tensor_add(negv[:], lg[:], mx[:])
        nc.sync.dma_start(out=neg_out[:, :], in_=negv[:])

        lo = state["out_lo"]
        nc.sync.dma_start(out=tc_out[:, lo:SLOTS, :], in_=tc_strip[:, lo:SLOTS, :])

    nc.compile()
    return nc


# ---------------------------------------------------------------------------
# Host-side driver: sharding, run, unskew, backtrack walk, final loss
# ---------------------------------------------------------------------------
import numpy as np

B_TOT, N_G, M_G, D_G = 64, 512, 512, 256
N_CORES = 8
B_LOC = B_TOT // N_CORES
S_G, W_G, R_G = 16, 32, 8
P_G = S_G * B_LOC
SLOTS_G = N_G + R_G * S_G
SLOT_G = W_G + 1

_NC_CACHE = {}


def _get_nc():
    if "nc" not in _NC_CACHE:
        _NC_CACHE["nc"] = _build_cfg(B=B_LOC, N=N_G, M=M_G, D=D_G,
                                     S=S_G, W=W_G, R=R_G)
    return _NC_CACHE["nc"]


def _unskew(tc_skew):
    tc = np.empty((B_LOC, N_G, M_G), np.float32)
    for s in range(S_G):
        for b in range(B_LOC):
            tc[b, :, s * W_G:(s + 1) * W_G] = \
                tc_skew[s * B_LOC + b, R_G * s:R_G * s + N_G, 1:SLOT_G]
    return tc


def _host_finish(tc, x, y, neg):
    """Backtrack walk on the device tc + pos logsumexp (host side)."""
    Bt, Nn, Mm = tc.shape
    eps = 1e-8
    xn = x / np.maximum(np.linalg.norm(x, axis=-1, keepdims=True), eps)
    yn = y / np.maximum(np.linalg.norm(y, axis=-1, keepdims=True), eps)
    bidx = np.arange(Bt)
    i = np.full(Bt, Nn - 1, np.int64)
    j = np.full(Bt, Mm - 1, np.int64)
    Is, Js, Vs = [i.copy()], [j.copy()], [np.ones(Bt, bool)]
    active = (i > 0) & (j > 0)
    while active.any():
        a = tc[bidx, np.maximum(i - 1, 0), np.maximum(j - 1, 0)]
        bb = tc[bidx, np.maximum(i - 1, 0), j]
        c = tc[bidx, i, np.maximum(j - 1, 0)]
        diag = (a <= bb) & (a <= c)
        up = (~diag) & (bb <= c)
        ni = np.where(diag | up, i - 1, i)
        nj = np.where(diag | (~up), j - 1, j)
        i = np.where(active, ni, i)
        j = np.where(active, nj, j)
        Is.append(i.copy())
        Js.append(j.copy())
        Vs.append(active.copy())
        active = (i > 0) & (j > 0)
    at00 = (i == 0) & (j == 0)
    Is.append(np.zeros(Bt, np.int64))
    Js.append(np.zeros(Bt, np.int64))
    Vs.append(~at00)

    IS = np.stack(Is, 1)
    JS = np.stack(Js, 1)
    VS = np.stack(Vs, 1)
    costs = 1.0 - np.einsum("bld,bld->bl",
                            xn[bidx[:, None], IS], yn[bidx[:, None], JS])
    colsum = np.zeros((Bt, Mm), np.float32)
    np.add.at(colsum, (bidx[:, None], JS),
              np.where(VS, costs, 0.0).astype(np.float32))
    mxv = colsum.max(axis=1, keepdims=True)
    pos = (mxv + np.log(np.sum(np.exp(colsum - mxv),
                               axis=1, keepdims=True))).squeeze(1)
    return (pos.astype(np.float32) - neg).astype(np.float32)


def run_device(x, y, **kw):
    from concourse import bass_utils

    nc = _get_nc()
    in_maps = [
        {"x": np.ascontiguousarray(x[c * B_LOC:(c + 1) * B_LOC]),
         "y": np.ascontiguousarray(y[c * B_LOC:(c + 1) * B_LOC])}
        for c in range(N_CORES)
    ]
    res = bass_utils.run_bass_kernel_spmd(nc, in_maps, list(range(N_CORES)), **kw)
    tc = np.empty((B_TOT, N_G, M_G), np.float32)
    neg = np.empty(B_TOT, np.float32)
    for c in range(N_CORES):
        out = res.results[c]
        tc[c * B_LOC:(c + 1) * B_LOC] = _unskew(out["tc_out"])
        neg[c * B_LOC:(c + 1) * B_LOC] = out["neg_out"].reshape(B_LOC)
    return tc, neg, res


def kernel(x, y):
    x = np.asarray(x, dtype=np.float32)
    y = np.asarray(y, dtype=np.float32)
    tc, neg, _ = run_device(x, y)
    return _host_finish(tc, x, y, neg)
